# revision 4
# baseline (speedup 1.0000x reference)
"""CFConv Trainium2 kernel.

Math: out[b,o,y,x] = sum_{k,i,j} weight[k,o,i,j] * fa[b,i,y+dy,x+dx] * wa[b,j,y+dy,x+dx]
(3x3 valid conv over the outer-product channel space of fa (65ch) x wa (17ch)).

Strategy (8 NeuronCores, SPMD):
- Shard (batch b, row-half h): each core computes 63 output rows of one batch.
- On-chip, form z[(i,j), pix] = f_i * w_j for the 64x16 "main" (i,j) grid as
  8 partition-chunks of 128 (j-major within 16-partition groups). The
  replicated-f factor is prepared host-side (np.repeat) and DMA'd per
  (pair, chunk) window, so the vector engine only does the elementwise
  multiply against a pre-tiled copy of w. The remaining 81 channels
  (j=16 ones column, i=64 ones row, corner) are read directly from a
  packed [f; w; ones] tensor.
- Contract with the tensor engine in fp16 (fp32 PSUM accumulation). Matmuls
  are issued in column-tiled pairs (tile_position (0,0)/(0,64)): the two
  64-wide PE column groups concurrently compute two adjacent 512-pixel output
  tiles, accumulating into the lower/upper partition halves of one PSUM bank.
- Output layout stays at input width (128) so all 9 conv offsets are plain
  column shifts; the two garbage columns per row are skipped at DMA-out.
- PSUM->SBUF staging runs on the scalar (Act) engine; warmup matmuls (HAM
  clock ramp) read a memset tile so they need no DMA.
"""

import numpy as np

B, WCH, FCH, OCH, H, W = 4, 16, 64, 64, 128, 128
KX = 3
HO = WO = H - KX + 1          # 126
ROWS_OUT = 63                 # output rows per core
ROWS_IN = 65                  # input rows per core
FREE = 8448                   # padded region width (66 rows * 128)
VALID = ROWS_IN * W           # 8320
NPAIR = 8                     # pixel-tile pairs per core
HALO = 2 * W + 2              # 258
WIN = 1024 + HALO             # 1282: z window per pair

_cache = {}


def _pair_c0(a):
    # pair 7 overlaps pair 6 by one row (cols 7040..8063) so every matmul
    # is a full N=512; the duplicated row is not stored.
    return 1024 * a if a < NPAIR - 1 else 7040


def _build_program():
    import concourse.bacc as bacc
    import concourse.mybir as mybir
    import concourse.tile as tile

    f16 = mybir.dt.float16
    f32 = mybir.dt.float32

    nc = bacc.Bacc("TRN2", target_bir_lowering=False)
    fw_d = nc.dram_tensor("fw", (81, FREE), f16, kind="ExternalInput")
    fpw_d = nc.dram_tensor("fpw", (128, NPAIR * 8 * WIN), f16, kind="ExternalInput")
    wt_d = nc.dram_tensor("wt", (128, FREE), f16, kind="ExternalInput")
    wkm_d = nc.dram_tensor("wkm", (128, 9 * 8 * 64), f16, kind="ExternalInput")
    wkx_d = nc.dram_tensor("wkx", (81, 9 * 64), f16, kind="ExternalInput")
    out_d = nc.dram_tensor("out", (OCH, ROWS_OUT, WO), f32, kind="ExternalOutput")

    with tile.TileContext(nc) as tc:
        with tc.tile_pool(name="inp", bufs=1) as inp, \
             tc.tile_pool(name="frep", bufs=2) as frp, \
             tc.tile_pool(name="z", bufs=3) as zp, \
             tc.tile_pool(name="st", bufs=3) as stp, \
             tc.tile_pool(name="ps", bufs=4, space="PSUM") as psp:
            # dummy matmuls warm the PE clock (HAM) while the input DMAs
            # land; fed by a memset tile (no DMA dependency), their PSUM
            # bank is never read.
            warm = inp.tile([128, 256], f16)
            nc.gpsimd.memset(warm[:], 0.0)
            warm_ps = psp.tile([128, 512], f32)
            for _ in range(18):
                nc.tensor.matmul(warm_ps[0:64, 0:256], warm[:, 0:64], warm[:, 0:256],
                                 start=True, stop=True, tile_position=(0, 0))

            fw_s = inp.tile([81, FREE], f16)
            wt_s = inp.tile([128, FREE], f16)
            wkm_s = inp.tile([128, 9 * 8 * 64], f16)
            wkx_s = inp.tile([81, 9 * 64], f16)

            # Queue discipline: DMA queues are strictly in-order and ramp
            # slowly for their first few us, so the head-of-line order is
            # everything. sync + gpsimd (+vector for the first pair) carry
            # the critical path in need-order; the slow scalar queue gets
            # only late-needed bulk and the output stream.
            first = 1344
            nc.sync.dma_start(fw_s[:, 0:first], fw_d[:, 0:first])
            nc.gpsimd.dma_start(wt_s[:, 0:first], wt_d[:, 0:first])
            nc.gpsimd.dma_start(wkx_s[:], wkx_d[:])
            nc.gpsimd.dma_start(wkm_s[:], wkm_d[:])
            # wt/fw remainder: chunk 1 needed by pair 1, chunk 2 by pair 3,
            # chunk 3 by pair 5/6 — the last two can live on the slow queue.
            cw = (FREE - first) // 3
            sl1 = slice(first, first + cw)
            sl2 = slice(first + cw, first + 2 * cw)
            sl3 = slice(first + 2 * cw, FREE)

            for a in range(NPAIR):
                c0 = _pair_c0(a)
                zs = []
                for c in range(8):
                    fr = frp.tile([128, WIN], f16, tag=f"fc{c}")
                    src = fpw_d[:, (a * 8 + c) * WIN:(a * 8 + c + 1) * WIN]
                    (nc.sync, nc.gpsimd)[c % 2].dma_start(fr[:], src)
                    z = zp.tile([128, WIN], f16, tag=f"z{c}")
                    nc.vector.tensor_mul(z[:], fr[:], wt_s[:, c0:c0 + WIN])
                    zs.append(z)
                if a == 0:
                    nc.sync.dma_start(fw_s[:, sl1], fw_d[:, sl1])
                    nc.gpsimd.dma_start(wt_s[:, sl1], wt_d[:, sl1])
                    nc.scalar.dma_start(wt_s[:, sl3], wt_d[:, sl3])
                    nc.scalar.dma_start(fw_s[:, sl3], fw_d[:, sl3])
                elif a == 1:
                    nc.sync.dma_start(fw_s[:, sl2], fw_d[:, sl2])
                    nc.gpsimd.dma_start(wt_s[:, sl2], wt_d[:, sl2])

                ps = psp.tile([128, 512], f32)
                for c in (8, 0, 1, 2, 3, 4, 5, 6, 7):
                    for k in range(9):
                        dy, dx = divmod(k, KX)
                        d = dy * W + dx
                        for g, off in ((0, 0), (1, 512)):
                            if c < 8:
                                lhsT = wkm_s[:, (c * 9 + k) * 64:(c * 9 + k) * 64 + 64]
                                rhs = zs[c][:, d + off:d + off + 512]
                            else:
                                lhsT = wkx_s[:, k * 64:k * 64 + 64]
                                rhs = fw_s[:, c0 + d + off:c0 + d + off + 512]
                            nc.tensor.matmul(
                                ps[64 * g:64 * g + 64, 0:512], lhsT, rhs,
                                start=(c == 8 and k == 0),
                                stop=(c == 7 and k == 8),
                                tile_position=(0, 64 * g),
                            )

                stage = stp.tile([128, 512], f32)
                nc.scalar.copy(stage[:], ps[:])
                for g in (0, 1):
                    if a < NPAIR - 1:
                        r_dst, col_lo, nrow = 8 * a + 4 * g, 0, 4
                    elif g == 0:
                        r_dst, col_lo, nrow = 56, 128, 3   # drop duplicated row 55
                    else:
                        r_dst, col_lo, nrow = 59, 0, 4
                    src = stage[64 * g:64 * g + 64, col_lo:col_lo + nrow * W].rearrange(
                        "p (r c) -> p r c", c=W)[:, :, 0:WO]
                    nc.scalar.dma_start(out_d[:, r_dst:r_dst + nrow, :], src)

    nc.finalize()
    return nc


def _prep_core(inputf, inputw, b, h):
    r0 = 63 * h
    f_reg = np.zeros((64, FREE), np.float16)
    f_reg[:, :VALID] = inputf[b, :, r0:r0 + ROWS_IN, :].reshape(64, VALID)
    w_reg = np.zeros((16, FREE), np.float16)
    w_reg[:, :VALID] = inputw[b, :, r0:r0 + ROWS_IN, :].reshape(16, VALID)
    ones_reg = np.zeros((1, FREE), np.float16)
    ones_reg[0, :VALID] = 1.0
    fw = np.concatenate([f_reg, w_reg, ones_reg], 0)

    # pre-replicated f windows, pair-major: chunk c rows are f_{8c..8c+8}
    # each repeated 16x; window [c0, c0+WIN) per pair.
    fpw = np.empty((128, NPAIR * 8 * WIN), np.float16)
    for c in range(8):
        frep = np.repeat(f_reg[8 * c:8 * c + 8], 16, axis=0)  # [128, FREE]
        for a in range(NPAIR):
            c0 = _pair_c0(a)
            fpw[:, (a * 8 + c) * WIN:(a * 8 + c + 1) * WIN] = frep[:, c0:c0 + WIN]

    wt = np.empty((128, FREE), np.float16)
    for u in range(8):
        wt[16 * u:16 * u + 16] = w_reg
    return fw, fpw, wt


def kernel(inputw, inputf, weight):
    from concourse import bass_utils

    inputw = np.asarray(inputw, np.float32)
    inputf = np.asarray(inputf, np.float32)
    weight = np.asarray(weight, np.float32)

    if "nc" not in _cache:
        _cache["nc"] = _build_program()
    nc = _cache["nc"]

    # weight layouts (replicated across cores)
    p = np.arange(128)
    wkm = np.empty((128, 8, 9, 64), np.float16)
    for t in range(8):
        iw = 8 * t + p // 16
        jw = p % 16
        wkm[:, t, :, :] = weight[:, :, iw, jw].transpose(2, 0, 1)
    wkm = wkm.reshape(128, 8 * 9 * 64)
    wkx = np.empty((81, 9, 64), np.float16)
    wkx[:64] = weight[:, :, :64, 16].transpose(2, 0, 1)
    wkx[64:80] = weight[:, :, 64, :16].transpose(2, 0, 1)
    wkx[80] = weight[:, :, 64, 16]
    wkx = wkx.reshape(81, 9 * 64)

    in_maps = []
    for core in range(8):
        b, h = divmod(core, 2)
        fw, fpw, wt = _prep_core(inputf, inputw, b, h)
        in_maps.append({"fw": fw, "fpw": fpw, "wt": wt, "wkm": wkm, "wkx": wkx})

    res = bass_utils.run_bass_kernel_spmd(nc, in_maps, core_ids=list(range(8)))
    kernel.last_result = res

    out = np.empty((B, OCH, HO, WO), np.float32)
    for core in range(8):
        b, h = divmod(core, 2)
        out[b, :, 63 * h:63 * h + 63, :] = res.results[core]["out"]
    return out


# revision 6
# speedup vs baseline: 1.0302x; 1.0302x over previous
"""CFConv Trainium2 kernel.

Math: out[b,o,y,x] = sum_{k,i,j} weight[k,o,i,j] * fa[b,i,y+dy,x+dx] * wa[b,j,y+dy,x+dx]
(3x3 valid conv over the outer-product channel space of fa (65ch) x wa (17ch)).

Strategy (8 NeuronCores, SPMD):
- Shard (batch b, row-half h): each core computes 63 output rows of one batch.
- On-chip, form z[(i,j), pix] = f_i * w_j for the 64x16 "main" (i,j) grid as
  8 partition-chunks of 128 (j-major within 16-partition groups). The
  replicated-f factor is prepared host-side (np.repeat) and DMA'd per
  (pair, chunk) window, so the vector engine only does the elementwise
  multiply against a pre-tiled copy of w. The remaining 81 channels
  (j=16 ones column, i=64 ones row, corner) are read directly from a
  packed [f; w; ones] tensor.
- Contract with the tensor engine in fp16 (fp32 PSUM accumulation). Matmuls
  are issued in column-tiled pairs (tile_position (0,0)/(0,64)): the two
  64-wide PE column groups concurrently compute two adjacent 512-pixel output
  tiles, accumulating into the lower/upper partition halves of one PSUM bank.
- Output layout stays at input width (128) so all 9 conv offsets are plain
  column shifts; the two garbage columns per row are skipped at DMA-out.
- PSUM->SBUF staging runs on the scalar (Act) engine; warmup matmuls (HAM
  clock ramp) read a memset tile so they need no DMA.
"""

import numpy as np

B, WCH, FCH, OCH, H, W = 4, 16, 64, 64, 128, 128
KX = 3
HO = WO = H - KX + 1          # 126
ROWS_OUT = 63                 # output rows per core
ROWS_IN = 65                  # input rows per core
FREE = 8448                   # padded region width (66 rows * 128)
VALID = ROWS_IN * W           # 8320
NPAIR = 8                     # pixel-tile pairs per core
HALO = 2 * W + 2              # 258
WIN = 1024 + HALO             # 1282: z window per pair

_cache = {}


def _pair_c0(a):
    # pair 7 overlaps pair 6 by one row (cols 7040..8063) so every matmul
    # is a full N=512; the duplicated row is not stored.
    return 1024 * a if a < NPAIR - 1 else 7040


def _build_program():
    import concourse.bacc as bacc
    import concourse.mybir as mybir
    import concourse.tile as tile

    f16 = mybir.dt.float16
    f32 = mybir.dt.float32

    nc = bacc.Bacc("TRN2", target_bir_lowering=False)
    fw_d = nc.dram_tensor("fw", (81, FREE), f16, kind="ExternalInput")
    fpw_d = nc.dram_tensor("fpw", (128, NPAIR * 8 * WIN), f16, kind="ExternalInput")
    wt_d = nc.dram_tensor("wt", (128, FREE), f16, kind="ExternalInput")
    wkm_d = nc.dram_tensor("wkm", (128, 9 * 8 * 64), f16, kind="ExternalInput")
    wkx_d = nc.dram_tensor("wkx", (81, 9 * 64), f16, kind="ExternalInput")
    out_d = nc.dram_tensor("out", (OCH, ROWS_OUT, WO), f32, kind="ExternalOutput")

    with tile.TileContext(nc) as tc:
        with tc.tile_pool(name="inp", bufs=1) as inp, \
             tc.tile_pool(name="frep", bufs=2) as frp, \
             tc.tile_pool(name="z", bufs=3) as zp, \
             tc.tile_pool(name="st", bufs=3) as stp, \
             tc.tile_pool(name="ps", bufs=4, space="PSUM") as psp:
            # dummy matmuls warm the PE clock (HAM) while the input DMAs
            # land; fed by a memset tile (no DMA dependency), their PSUM
            # bank is never read.
            warm = inp.tile([128, 256], f16)
            nc.gpsimd.memset(warm[:], 0.0)
            warm_ps = psp.tile([128, 512], f32)
            for _ in range(42):
                nc.tensor.matmul(warm_ps[0:64, 0:256], warm[:, 0:64], warm[:, 0:256],
                                 start=True, stop=True, tile_position=(0, 0))

            fw_s = inp.tile([81, FREE], f16)
            wt_s = inp.tile([128, FREE], f16)
            wkm_s = inp.tile([128, 9 * 8 * 64], f16)
            wkx_s = inp.tile([81, 9 * 64], f16)

            # Queue discipline: DMA rings are strictly in-order and the
            # ~300 GB/s aggregate is shared FCFS across rings, so only
            # in-stream position on a ring delays a transfer — program
            # position relative to other engines does not. Critical-path
            # windows go first on sync/gpsimd; bulk is placed in-stream
            # behind the early pairs' frep windows; the scalar ring only
            # carries the output stream.
            first = 1344
            nc.sync.dma_start(fw_s[:, 0:first], fw_d[:, 0:first])
            nc.gpsimd.dma_start(wt_s[:, 0:first], wt_d[:, 0:first])
            nc.gpsimd.dma_start(wkx_s[:], wkx_d[:])
            nc.gpsimd.dma_start(wkm_s[:], wkm_d[:])
            cw = (FREE - first) // 3
            sls = [slice(first + i * cw, first + (i + 1) * cw if i < 2 else FREE)
                   for i in range(3)]
            # (engine, dst, src, slice) bulk pieces, interleaved in-stream
            # after pair 0/1/2's frep windows respectively.
            bulk = [
                [(nc.sync, fw_s, fw_d, sls[0]), (nc.gpsimd, wt_s, wt_d, sls[0])],
                [(nc.sync, fw_s, fw_d, sls[1]), (nc.gpsimd, wt_s, wt_d, sls[1])],
                [(nc.sync, fw_s, fw_d, sls[2]), (nc.gpsimd, wt_s, wt_d, sls[2])],
            ]

            for a in range(NPAIR):
                c0 = _pair_c0(a)
                zs = []
                for c in range(8):
                    fr = frp.tile([128, WIN], f16, tag=f"fc{c}")
                    src = fpw_d[:, (a * 8 + c) * WIN:(a * 8 + c + 1) * WIN]
                    (nc.sync, nc.gpsimd)[c % 2].dma_start(fr[:], src)
                    z = zp.tile([128, WIN], f16, tag=f"z{c}")
                    nc.vector.tensor_mul(z[:], fr[:], wt_s[:, c0:c0 + WIN])
                    zs.append(z)
                if a < len(bulk):
                    for eng, dst, srcd, sl in bulk[a]:
                        eng.dma_start(dst[:, sl], srcd[:, sl])

                ps = psp.tile([128, 512], f32)
                for c in (8, 0, 1, 2, 3, 4, 5, 6, 7):
                    for k in range(9):
                        dy, dx = divmod(k, KX)
                        d = dy * W + dx
                        for g, off in ((0, 0), (1, 512)):
                            if c < 8:
                                lhsT = wkm_s[:, (c * 9 + k) * 64:(c * 9 + k) * 64 + 64]
                                rhs = zs[c][:, d + off:d + off + 512]
                            else:
                                lhsT = wkx_s[:, k * 64:k * 64 + 64]
                                rhs = fw_s[:, c0 + d + off:c0 + d + off + 512]
                            nc.tensor.matmul(
                                ps[64 * g:64 * g + 64, 0:512], lhsT, rhs,
                                start=(c == 8 and k == 0),
                                stop=(c == 7 and k == 8),
                                tile_position=(0, 64 * g),
                            )

                stage = stp.tile([128, 512], f32)
                nc.scalar.copy(stage[:], ps[:])
                for g in (0, 1):
                    if a < NPAIR - 1:
                        r_dst, col_lo, nrow = 8 * a + 4 * g, 0, 4
                    elif g == 0:
                        r_dst, col_lo, nrow = 56, 128, 3   # drop duplicated row 55
                    else:
                        r_dst, col_lo, nrow = 59, 0, 4
                    src = stage[64 * g:64 * g + 64, col_lo:col_lo + nrow * W].rearrange(
                        "p (r c) -> p r c", c=W)[:, :, 0:WO]
                    nc.scalar.dma_start(out_d[:, r_dst:r_dst + nrow, :], src)

    nc.finalize()
    return nc


def _prep_core(inputf, inputw, b, h):
    r0 = 63 * h
    f_reg = np.zeros((64, FREE), np.float16)
    f_reg[:, :VALID] = inputf[b, :, r0:r0 + ROWS_IN, :].reshape(64, VALID)
    w_reg = np.zeros((16, FREE), np.float16)
    w_reg[:, :VALID] = inputw[b, :, r0:r0 + ROWS_IN, :].reshape(16, VALID)
    ones_reg = np.zeros((1, FREE), np.float16)
    ones_reg[0, :VALID] = 1.0
    fw = np.concatenate([f_reg, w_reg, ones_reg], 0)

    # pre-replicated f windows, pair-major: chunk c rows are f_{8c..8c+8}
    # each repeated 16x; window [c0, c0+WIN) per pair.
    fpw = np.empty((128, NPAIR * 8 * WIN), np.float16)
    for c in range(8):
        frep = np.repeat(f_reg[8 * c:8 * c + 8], 16, axis=0)  # [128, FREE]
        for a in range(NPAIR):
            c0 = _pair_c0(a)
            fpw[:, (a * 8 + c) * WIN:(a * 8 + c + 1) * WIN] = frep[:, c0:c0 + WIN]

    wt = np.empty((128, FREE), np.float16)
    for u in range(8):
        wt[16 * u:16 * u + 16] = w_reg
    return fw, fpw, wt


def kernel(inputw, inputf, weight):
    from concourse import bass_utils

    inputw = np.asarray(inputw, np.float32)
    inputf = np.asarray(inputf, np.float32)
    weight = np.asarray(weight, np.float32)

    if "nc" not in _cache:
        _cache["nc"] = _build_program()
    nc = _cache["nc"]

    # weight layouts (replicated across cores)
    p = np.arange(128)
    wkm = np.empty((128, 8, 9, 64), np.float16)
    for t in range(8):
        iw = 8 * t + p // 16
        jw = p % 16
        wkm[:, t, :, :] = weight[:, :, iw, jw].transpose(2, 0, 1)
    wkm = wkm.reshape(128, 8 * 9 * 64)
    wkx = np.empty((81, 9, 64), np.float16)
    wkx[:64] = weight[:, :, :64, 16].transpose(2, 0, 1)
    wkx[64:80] = weight[:, :, 64, :16].transpose(2, 0, 1)
    wkx[80] = weight[:, :, 64, 16]
    wkx = wkx.reshape(81, 9 * 64)

    in_maps = []
    for core in range(8):
        b, h = divmod(core, 2)
        fw, fpw, wt = _prep_core(inputf, inputw, b, h)
        in_maps.append({"fw": fw, "fpw": fpw, "wt": wt, "wkm": wkm, "wkx": wkx})

    res = bass_utils.run_bass_kernel_spmd(nc, in_maps, core_ids=list(range(8)))
    kernel.last_result = res

    out = np.empty((B, OCH, HO, WO), np.float32)
    for core in range(8):
        b, h = divmod(core, 2)
        out[b, :, 63 * h:63 * h + 63, :] = res.results[core]["out"]
    return out


# revision 10
# speedup vs baseline: 1.0409x; 1.0104x over previous
"""CFConv Trainium2 kernel.

Math: out[b,o,y,x] = sum_{k,i,j} weight[k,o,i,j] * fa[b,i,y+dy,x+dx] * wa[b,j,y+dy,x+dx]
(3x3 valid conv over the outer-product channel space of fa (65ch) x wa (17ch)).

Strategy (8 NeuronCores, SPMD):
- Shard (batch b, row-half h): each core computes 63 output rows of one batch.
- On-chip, form z[(i,j), pix] = f_i * w_j for the 64x16 "main" (i,j) grid as
  8 partition-chunks of 128 (j-major within 16-partition groups). The
  replicated-f factor is prepared host-side (np.repeat) and DMA'd per
  (pair, chunk) window, so the vector engine only does the elementwise
  multiply against a pre-tiled copy of w. The remaining 81 channels
  (j=16 ones column, i=64 ones row, corner) are read directly from a
  packed [f; w; ones] tensor.
- Contract with the tensor engine in fp16 (fp32 PSUM accumulation). Matmuls
  are issued in column-tiled pairs (tile_position (0,0)/(0,64)): the two
  64-wide PE column groups concurrently compute two adjacent 512-pixel output
  tiles, accumulating into the lower/upper partition halves of one PSUM bank.
- Output layout stays at input width (128) so all 9 conv offsets are plain
  column shifts; the two garbage columns per row are skipped at DMA-out.
- PSUM->SBUF staging runs on the scalar (Act) engine; warmup matmuls (HAM
  clock ramp) read a memset tile so they need no DMA.
"""

import numpy as np

B, WCH, FCH, OCH, H, W = 4, 16, 64, 64, 128, 128
KX = 3
HO = WO = H - KX + 1          # 126
ROWS_OUT = 63                 # output rows per core
ROWS_IN = 65                  # input rows per core
FREE = 8448                   # padded region width (66 rows * 128)
VALID = ROWS_IN * W           # 8320
NPAIR = 8                     # pixel-tile pairs per core
HALO = 2 * W + 2              # 258
WIN = 1024 + HALO             # 1282: z window per pair

_cache = {}


def _pair_c0(a):
    # pair 7 overlaps pair 6 by one row (cols 7040..8063) so every matmul
    # is a full N=512; the duplicated row is not stored.
    return 1024 * a if a < NPAIR - 1 else 7040


def _build_program():
    import concourse.bacc as bacc
    import concourse.mybir as mybir
    import concourse.tile as tile

    f16 = mybir.dt.float16
    f32 = mybir.dt.float32

    nc = bacc.Bacc("TRN2", target_bir_lowering=False)
    fw_d = nc.dram_tensor("fw", (81, FREE), f16, kind="ExternalInput")
    fpre_d = nc.dram_tensor("fpre", (128, WIN), f16, kind="ExternalInput")
    fpw_d = nc.dram_tensor("fpw", (128, NPAIR * 8 * WIN), f16, kind="ExternalInput")
    wt_d = nc.dram_tensor("wt", (128, FREE), f16, kind="ExternalInput")
    wkm_d = nc.dram_tensor("wkm", (128, 9 * 8 * 64), f16, kind="ExternalInput")
    wkx_d = nc.dram_tensor("wkx", (81, 9 * 64), f16, kind="ExternalInput")
    out_d = nc.dram_tensor("out", (OCH, ROWS_OUT, WO), f32, kind="ExternalOutput")

    with tile.TileContext(nc) as tc:
        with tc.tile_pool(name="inp", bufs=1) as inp, \
             tc.tile_pool(name="frep", bufs=2) as frp, \
             tc.tile_pool(name="z", bufs=3) as zp, \
             tc.tile_pool(name="st", bufs=3) as stp, \
             tc.tile_pool(name="ps", bufs=4, space="PSUM") as psp:
            # dummy matmuls warm the PE clock (HAM) while the input DMAs
            # land; fed by a memset tile (no DMA dependency), their PSUM
            # bank is never read.
            warm = inp.tile([128, 256], f16)
            nc.gpsimd.memset(warm[:], 0.0)
            warm_ps = psp.tile([128, 512], f32)
            for _ in range(26):
                nc.tensor.matmul(warm_ps[0:64, 0:256], warm[:, 0:64], warm[:, 0:256],
                                 start=True, stop=True, tile_position=(0, 0))

            fw_s = inp.tile([81, FREE], f16)
            fpre_s = inp.tile([128, WIN], f16)
            wt_s = inp.tile([128, FREE], f16)
            wkm_s = inp.tile([128, 9 * 8 * 64], f16)
            wkx_s = inp.tile([81, 9 * 64], f16)

            # Queue discipline: DMA rings are strictly in-order and the
            # ~250-300 GB/s aggregate is shared FCFS across rings (each
            # ring sustains ~90-170 GB/s), so only in-stream position on
            # a ring delays a transfer. The head carries only what the
            # first pair needs — pair 0's replicated-f factor is built by
            # stream_shuffle from one small fpre window instead of eight
            # DMA windows; DMA-fed frep starts at pair 1. Bulk width is
            # threaded in-stream behind the early pairs' windows.
            first = 1344
            half = 672
            nc.sync.dma_start(fw_s[:, 0:first], fw_d[:, 0:first])
            nc.sync.dma_start(fpre_s[:, 0:half], fpre_d[:, 0:half])
            nc.sync.dma_start(wt_s[:, 0:half], wt_d[:, 0:half])
            nc.gpsimd.dma_start(wkx_s[:], wkx_d[:])
            nc.gpsimd.dma_start(fpre_s[:, half:WIN], fpre_d[:, half:WIN])
            nc.gpsimd.dma_start(wt_s[:, half:first], wt_d[:, half:first])
            nc.scalar.dma_start(wkm_s[:], wkm_d[:])
            cw = (FREE - first) // 3
            sls = [slice(first + i * cw, first + (i + 1) * cw if i < 2 else FREE)
                   for i in range(3)]

            for a in range(NPAIR):
                c0 = _pair_c0(a)
                zs = []
                for c in range(8):
                    fr = frp.tile([128, WIN], f16, tag=f"fc{c}")
                    if a == 0:
                        mask = [2 * c + (r // 16) for r in range(32)]
                        nc.vector.stream_shuffle(fr[:], fpre_s[:], mask)
                    else:
                        src = fpw_d[:, (a * 8 + c) * WIN:(a * 8 + c + 1) * WIN]
                        (nc.sync, nc.gpsimd)[c % 2].dma_start(fr[:], src)
                    z = zp.tile([128, WIN], f16, tag=f"z{c}")
                    nc.vector.tensor_mul(z[:], fr[:], wt_s[:, c0:c0 + WIN])
                    zs.append(z)
                # bulk width for pairs a+1.. threads in behind this pair
                if a == 0:
                    nc.sync.dma_start(fw_s[:, sls[0]], fw_d[:, sls[0]])
                    nc.gpsimd.dma_start(wt_s[:, sls[0]], wt_d[:, sls[0]])
                elif a == 1:
                    nc.sync.dma_start(fw_s[:, sls[1]], fw_d[:, sls[1]])
                    nc.gpsimd.dma_start(wt_s[:, sls[1]], wt_d[:, sls[1]])
                elif a == 2:
                    nc.sync.dma_start(fw_s[:, sls[2]], fw_d[:, sls[2]])
                    nc.gpsimd.dma_start(wt_s[:, sls[2]], wt_d[:, sls[2]])

                ps = psp.tile([128, 512], f32)
                for c in (8, 0, 1, 2, 3, 4, 5, 6, 7):
                    for k in range(9):
                        dy, dx = divmod(k, KX)
                        d = dy * W + dx
                        for g, off in ((0, 0), (1, 512)):
                            if c < 8:
                                lhsT = wkm_s[:, (c * 9 + k) * 64:(c * 9 + k) * 64 + 64]
                                rhs = zs[c][:, d + off:d + off + 512]
                            else:
                                lhsT = wkx_s[:, k * 64:k * 64 + 64]
                                rhs = fw_s[:, c0 + d + off:c0 + d + off + 512]
                            nc.tensor.matmul(
                                ps[64 * g:64 * g + 64, 0:512], lhsT, rhs,
                                start=(c == 8 and k == 0),
                                stop=(c == 7 and k == 8),
                                tile_position=(0, 64 * g),
                            )

                stage = stp.tile([128, 512], f32)
                nc.scalar.copy(stage[:], ps[:])
                for g in (0, 1):
                    if a < NPAIR - 1:
                        r_dst, col_lo, nrow = 8 * a + 4 * g, 0, 4
                    elif g == 0:
                        r_dst, col_lo, nrow = 56, 128, 3   # drop duplicated row 55
                    else:
                        r_dst, col_lo, nrow = 59, 0, 4
                    src = stage[64 * g:64 * g + 64, col_lo:col_lo + nrow * W].rearrange(
                        "p (r c) -> p r c", c=W)[:, :, 0:WO]
                    nc.scalar.dma_start(out_d[:, r_dst:r_dst + nrow, :], src)

    nc.finalize()
    return nc


def _prep_core(inputf, inputw, b, h):
    r0 = 63 * h
    f_reg = np.zeros((64, FREE), np.float16)
    f_reg[:, :VALID] = inputf[b, :, r0:r0 + ROWS_IN, :].reshape(64, VALID)
    w_reg = np.zeros((16, FREE), np.float16)
    w_reg[:, :VALID] = inputw[b, :, r0:r0 + ROWS_IN, :].reshape(16, VALID)
    ones_reg = np.zeros((1, FREE), np.float16)
    ones_reg[0, :VALID] = 1.0
    fw = np.concatenate([f_reg, w_reg, ones_reg], 0)

    # pre-replicated f windows, pair-major: chunk c rows are f_{8c..8c+8}
    # each repeated 16x; window [c0, c0+WIN) per pair (pairs 1.. only —
    # pair 0 is built on-chip by stream_shuffle from fpre).
    fpw = np.empty((128, NPAIR * 8 * WIN), np.float16)
    for c in range(8):
        frep = np.repeat(f_reg[8 * c:8 * c + 8], 16, axis=0)  # [128, FREE]
        for a in range(1, NPAIR):
            c0 = _pair_c0(a)
            fpw[:, (a * 8 + c) * WIN:(a * 8 + c + 1) * WIN] = frep[:, c0:c0 + WIN]

    # shuffle-source layout for pair 0 (quadrant-permuted f rows)
    fpre = np.zeros((128, WIN), np.float16)
    q = np.arange(4)[:, None]
    s = np.arange(16)[None, :]
    rows = (8 * (s // 2) + 2 * q + (s % 2)).reshape(-1)
    idx = (32 * q + s).reshape(-1)
    fpre[idx] = f_reg[rows, 0:WIN]

    wt = np.empty((128, FREE), np.float16)
    for u in range(8):
        wt[16 * u:16 * u + 16] = w_reg
    return fw, fpre, fpw, wt


def kernel(inputw, inputf, weight):
    from concourse import bass_utils

    inputw = np.asarray(inputw, np.float32)
    inputf = np.asarray(inputf, np.float32)
    weight = np.asarray(weight, np.float32)

    if "nc" not in _cache:
        _cache["nc"] = _build_program()
    nc = _cache["nc"]

    # weight layouts (replicated across cores)
    p = np.arange(128)
    wkm = np.empty((128, 8, 9, 64), np.float16)
    for t in range(8):
        iw = 8 * t + p // 16
        jw = p % 16
        wkm[:, t, :, :] = weight[:, :, iw, jw].transpose(2, 0, 1)
    wkm = wkm.reshape(128, 8 * 9 * 64)
    wkx = np.empty((81, 9, 64), np.float16)
    wkx[:64] = weight[:, :, :64, 16].transpose(2, 0, 1)
    wkx[64:80] = weight[:, :, 64, :16].transpose(2, 0, 1)
    wkx[80] = weight[:, :, 64, 16]
    wkx = wkx.reshape(81, 9 * 64)

    in_maps = []
    for core in range(8):
        b, h = divmod(core, 2)
        fw, fpre, fpw, wt = _prep_core(inputf, inputw, b, h)
        in_maps.append({"fw": fw, "fpre": fpre, "fpw": fpw, "wt": wt,
                        "wkm": wkm, "wkx": wkx})

    res = bass_utils.run_bass_kernel_spmd(nc, in_maps, core_ids=list(range(8)))
    kernel.last_result = res

    out = np.empty((B, OCH, HO, WO), np.float32)
    for core in range(8):
        b, h = divmod(core, 2)
        out[b, :, 63 * h:63 * h + 63, :] = res.results[core]["out"]
    return out


# revision 13
# speedup vs baseline: 1.0451x; 1.0041x over previous
"""CFConv Trainium2 kernel.

Math: out[b,o,y,x] = sum_{k,i,j} weight[k,o,i,j] * fa[b,i,y+dy,x+dx] * wa[b,j,y+dy,x+dx]
(3x3 valid conv over the outer-product channel space of fa (65ch) x wa (17ch)).

Strategy (8 NeuronCores, SPMD):
- Shard (batch b, row-half h): each core computes 63 output rows of one batch.
- On-chip, form z[(i,j), pix] = f_i * w_j for the 64x16 "main" (i,j) grid as
  8 partition-chunks of 128 (j-major within 16-partition groups). The
  replicated-f factor is prepared host-side (np.repeat) and DMA'd per
  (pair, chunk) window, so the vector engine only does the elementwise
  multiply against a pre-tiled copy of w. The remaining 81 channels
  (j=16 ones column, i=64 ones row, corner) are read directly from a
  packed [f; w; ones] tensor.
- Contract with the tensor engine in fp16 (fp32 PSUM accumulation). Matmuls
  are issued in column-tiled pairs (tile_position (0,0)/(0,64)): the two
  64-wide PE column groups concurrently compute two adjacent 512-pixel output
  tiles, accumulating into the lower/upper partition halves of one PSUM bank.
- Output layout stays at input width (128) so all 9 conv offsets are plain
  column shifts; the two garbage columns per row are skipped at DMA-out.
- PSUM->SBUF staging runs on the scalar (Act) engine; warmup matmuls (HAM
  clock ramp) read a memset tile so they need no DMA.
"""

import numpy as np

B, WCH, FCH, OCH, H, W = 4, 16, 64, 64, 128, 128
KX = 3
HO = WO = H - KX + 1          # 126
ROWS_OUT = 63                 # output rows per core
ROWS_IN = 65                  # input rows per core
FREE = 8448                   # padded region width (66 rows * 128)
VALID = ROWS_IN * W           # 8320
NPAIR = 8                     # pixel-tile pairs per core
HALO = 2 * W + 2              # 258
WIN = 1024 + HALO             # 1282: z window per pair

_cache = {}


def _pair_c0(a):
    # pair 7 overlaps pair 6 by one row (cols 7040..8063) so every matmul
    # is a full N=512; the duplicated row is not stored.
    return 1024 * a if a < NPAIR - 1 else 7040


def _build_program():
    import concourse.bacc as bacc
    import concourse.mybir as mybir
    import concourse.tile as tile

    f16 = mybir.dt.float16
    f32 = mybir.dt.float32

    nc = bacc.Bacc("TRN2", target_bir_lowering=False)
    fw_d = nc.dram_tensor("fw", (81, FREE), f16, kind="ExternalInput")
    fpre_d = nc.dram_tensor("fpre", (128, FREE), f16, kind="ExternalInput")
    fpw_d = nc.dram_tensor("fpw", (128, NPAIR * 4 * WIN), f16, kind="ExternalInput")
    wt_d = nc.dram_tensor("wt", (128, FREE), f16, kind="ExternalInput")
    wkm_d = nc.dram_tensor("wkm", (128, 9 * 8 * 64), f16, kind="ExternalInput")
    wkx_d = nc.dram_tensor("wkx", (81, 9 * 64), f16, kind="ExternalInput")
    out_d = nc.dram_tensor("out", (OCH, ROWS_OUT, WO), f32, kind="ExternalOutput")

    with tile.TileContext(nc) as tc:
        with tc.tile_pool(name="inp", bufs=1) as inp, \
             tc.tile_pool(name="frep", bufs=2) as frp, \
             tc.tile_pool(name="z", bufs=3) as zp, \
             tc.tile_pool(name="st", bufs=3) as stp, \
             tc.tile_pool(name="ps", bufs=4, space="PSUM") as psp:
            # dummy matmuls warm the PE clock (HAM) while the input DMAs
            # land; fed by a memset tile (no DMA dependency), their PSUM
            # bank is never read.
            warm = inp.tile([128, 256], f16)
            nc.gpsimd.memset(warm[:], 0.0)
            warm_ps = psp.tile([128, 512], f32)
            for _ in range(26):
                nc.tensor.matmul(warm_ps[0:64, 0:256], warm[:, 0:64], warm[:, 0:256],
                                 start=True, stop=True, tile_position=(0, 0))

            fw_s = inp.tile([81, FREE], f16)
            fpre_s = inp.tile([128, FREE], f16)
            wt_s = inp.tile([128, FREE], f16)
            wkm_s = inp.tile([128, 9 * 8 * 64], f16)
            wkx_s = inp.tile([81, 9 * 64], f16)

            # DMA budget: all dynamic rings share one DMA engine and the 8
            # cores share the device fabric (~150 GB/s/core sustained), so
            # the replicated-f factor is DMA-fed for chunks 4-7 only;
            # chunks 0-3 are built by stream_shuffle on DVE (which then
            # runs ~73% busy). Rings are strictly in-order: the head
            # carries only pair 0's needs, bulk width threads in behind
            # the early pairs in need-order.
            first = 1344
            half = 672
            nc.sync.dma_start(fw_s[:, 0:first], fw_d[:, 0:first])
            nc.sync.dma_start(fpre_s[:, 0:half], fpre_d[:, 0:half])
            nc.sync.dma_start(wt_s[:, 0:half], wt_d[:, 0:half])
            nc.gpsimd.dma_start(wkx_s[:], wkx_d[:])
            nc.gpsimd.dma_start(fpre_s[:, half:first], fpre_d[:, half:first])
            nc.gpsimd.dma_start(wt_s[:, half:first], wt_d[:, half:first])
            nc.scalar.dma_start(wkm_s[:], wkm_d[:])
            cw = (FREE - first) // 3
            sls = [slice(first + i * cw, first + (i + 1) * cw if i < 2 else FREE)
                   for i in range(3)]
            # (pair -> list of (engine, dst, src, slice)) bulk width pieces,
            # placed in-ring behind that pair's frep windows, in need-order.
            bulk = {
                0: [(nc.sync, wt_s, wt_d, sls[0]), (nc.gpsimd, fpre_s, fpre_d, sls[0]),
                    (nc.sync, fw_s, fw_d, sls[0])],
                1: [(nc.sync, wt_s, wt_d, sls[1]), (nc.gpsimd, fpre_s, fpre_d, sls[1])],
                2: [(nc.sync, fw_s, fw_d, sls[1])],
                3: [(nc.sync, wt_s, wt_d, sls[2]), (nc.gpsimd, fpre_s, fpre_d, sls[2])],
                4: [(nc.sync, fw_s, fw_d, sls[2])],
            }

            for a in range(NPAIR):
                c0 = _pair_c0(a)
                zs = []
                for c in range(8):
                    fr = frp.tile([128, WIN], f16, tag=f"fc{c}")
                    if c < 4:
                        mask = [2 * c + (r // 16) for r in range(32)]
                        nc.vector.stream_shuffle(fr[:], fpre_s[:, c0:c0 + WIN], mask)
                    else:
                        src = fpw_d[:, (a * 4 + c - 4) * WIN:(a * 4 + c - 3) * WIN]
                        (nc.sync, nc.gpsimd)[c % 2].dma_start(fr[:], src)
                    z = zp.tile([128, WIN], f16, tag=f"z{c}")
                    nc.vector.tensor_mul(z[:], fr[:], wt_s[:, c0:c0 + WIN])
                    zs.append(z)
                for eng, dst, srcd, sl in bulk.get(a, []):
                    eng.dma_start(dst[:, sl], srcd[:, sl])

                ps = psp.tile([128, 512], f32)
                for c in (8, 0, 1, 2, 3, 4, 5, 6, 7):
                    for k in range(9):
                        dy, dx = divmod(k, KX)
                        d = dy * W + dx
                        for g, off in ((0, 0), (1, 512)):
                            if c < 8:
                                lhsT = wkm_s[:, (c * 9 + k) * 64:(c * 9 + k) * 64 + 64]
                                rhs = zs[c][:, d + off:d + off + 512]
                            else:
                                lhsT = wkx_s[:, k * 64:k * 64 + 64]
                                rhs = fw_s[:, c0 + d + off:c0 + d + off + 512]
                            nc.tensor.matmul(
                                ps[64 * g:64 * g + 64, 0:512], lhsT, rhs,
                                start=(c == 8 and k == 0),
                                stop=(c == 7 and k == 8),
                                tile_position=(0, 64 * g),
                            )

                stage = stp.tile([128, 512], f32)
                nc.scalar.copy(stage[:], ps[:])
                for g in (0, 1):
                    if a < NPAIR - 1:
                        r_dst, col_lo, nrow = 8 * a + 4 * g, 0, 4
                    elif g == 0:
                        r_dst, col_lo, nrow = 56, 128, 3   # drop duplicated row 55
                    else:
                        r_dst, col_lo, nrow = 59, 0, 4
                    src = stage[64 * g:64 * g + 64, col_lo:col_lo + nrow * W].rearrange(
                        "p (r c) -> p r c", c=W)[:, :, 0:WO]
                    nc.scalar.dma_start(out_d[:, r_dst:r_dst + nrow, :], src)

    nc.finalize()
    return nc


def _prep_core(inputf, inputw, b, h):
    r0 = 63 * h
    f_reg = np.zeros((64, FREE), np.float16)
    f_reg[:, :VALID] = inputf[b, :, r0:r0 + ROWS_IN, :].reshape(64, VALID)
    w_reg = np.zeros((16, FREE), np.float16)
    w_reg[:, :VALID] = inputw[b, :, r0:r0 + ROWS_IN, :].reshape(16, VALID)
    ones_reg = np.zeros((1, FREE), np.float16)
    ones_reg[0, :VALID] = 1.0
    fw = np.concatenate([f_reg, w_reg, ones_reg], 0)

    # pre-replicated f windows for the DMA-fed chunks 4-7, pair-major:
    # chunk c rows are f_{8c..8c+8} each repeated 16x; window
    # [c0, c0+WIN) per pair. Chunks 0-3 are built by stream_shuffle.
    fpw = np.empty((128, NPAIR * 4 * WIN), np.float16)
    for c in range(4, 8):
        frep = np.repeat(f_reg[8 * c:8 * c + 8], 16, axis=0)  # [128, FREE]
        for a in range(NPAIR):
            c0 = _pair_c0(a)
            fpw[:, (a * 4 + c - 4) * WIN:(a * 4 + c - 3) * WIN] = frep[:, c0:c0 + WIN]

    # shuffle-source layout for chunks 0-3 (quadrant-permuted f rows)
    fpre = np.zeros((128, FREE), np.float16)
    q = np.arange(4)[:, None]
    s = np.arange(16)[None, :]
    rows = (8 * (s // 2) + 2 * q + (s % 2)).reshape(-1)
    idx = (32 * q + s).reshape(-1)
    fpre[idx] = f_reg[rows]

    wt = np.empty((128, FREE), np.float16)
    for u in range(8):
        wt[16 * u:16 * u + 16] = w_reg
    return fw, fpre, fpw, wt


def kernel(inputw, inputf, weight):
    from concourse import bass_utils

    inputw = np.asarray(inputw, np.float32)
    inputf = np.asarray(inputf, np.float32)
    weight = np.asarray(weight, np.float32)

    if "nc" not in _cache:
        _cache["nc"] = _build_program()
    nc = _cache["nc"]

    # weight layouts (replicated across cores)
    p = np.arange(128)
    wkm = np.empty((128, 8, 9, 64), np.float16)
    for t in range(8):
        iw = 8 * t + p // 16
        jw = p % 16
        wkm[:, t, :, :] = weight[:, :, iw, jw].transpose(2, 0, 1)
    wkm = wkm.reshape(128, 8 * 9 * 64)
    wkx = np.empty((81, 9, 64), np.float16)
    wkx[:64] = weight[:, :, :64, 16].transpose(2, 0, 1)
    wkx[64:80] = weight[:, :, 64, :16].transpose(2, 0, 1)
    wkx[80] = weight[:, :, 64, 16]
    wkx = wkx.reshape(81, 9 * 64)

    in_maps = []
    for core in range(8):
        b, h = divmod(core, 2)
        fw, fpre, fpw, wt = _prep_core(inputf, inputw, b, h)
        in_maps.append({"fw": fw, "fpre": fpre, "fpw": fpw, "wt": wt,
                        "wkm": wkm, "wkx": wkx})

    res = bass_utils.run_bass_kernel_spmd(nc, in_maps, core_ids=list(range(8)))
    kernel.last_result = res

    out = np.empty((B, OCH, HO, WO), np.float32)
    for core in range(8):
        b, h = divmod(core, 2)
        out[b, :, 63 * h:63 * h + 63, :] = res.results[core]["out"]
    return out
